# revision 3
# baseline (speedup 1.0000x reference)
"""Trainium2 Bass kernel for nn_Caps_Layer (capsule routing layer).

Reference computation (per batch b of 1024):
  u_hat[b] = (x[b] @ W).reshape(512, 5, 5) -> transpose to [5cap, 512, 5dim]
  4 rounds of routing:
    c = softmax_over_cap(blog); o = squash(sum_s c*u); blog = einsum(o, u)
  output: o [1024, 5, 5]

Sharding: pure data parallel over batch across 8 cores (128 batches/core).

Per-core design (layout "B": token-position on SBUF partitions):
  - x shard [65536, 120] streamed batch-by-batch; each batch [512, 120] lands
    in SBUF as [128p, 4*120] (partition p holds tokens 4p..4p+3 contiguously).
  - PE transpose (matmul is_transpose w/ identity) turns [128t, 120d] tiles
    into xT [120d, 128t] (PSUM), ACT copies them to SBUF.
  - GEMM: lhsT = xT tile (stationary), rhs = W [120, 25] -> u tile [128t, 25].
  - u1 in SBUF: [128p, (q:4, b:128, i:5, k:5)] where token s = 4p + q.
  - Routing entirely on-chip:
      sum_s via PE matmul with all-ones stationary [128, 128] -> result
      replicated across all 128 partitions (free partition-broadcast).
      softmax over i / squash / blog updates on DVE+ACT with broadcast APs.
  - Output [128b, 25] written from one replicated row.
"""

import numpy as np

NCORES = 8
B, S, D = 1024, 512, 120
NCAP, DCAP = 5, 5
IK = NCAP * DCAP  # 25
BC = B // NCORES  # 128 batches per core
TOK = BC * S      # 65536 tokens per core
EPS = 1e-7
ROUTINGS = 4

QB = 4            # batches per GEMM psum group (quad)
NQUAD = BC // QB  # 32
CHUNK = 16        # batches per routing Σ_s psum chunk
NCHUNK = BC // CHUNK  # 8


def _build():
    import concourse.bass as bass
    import concourse.bacc as bacc
    import concourse.tile as tile
    from concourse import mybir
    from concourse.masks import make_identity

    f32 = mybir.dt.float32
    AF = mybir.ActivationFunctionType
    ALU = mybir.AluOpType

    nc = bacc.Bacc("TRN2", target_bir_lowering=False, debug=False)
    x_d = nc.dram_tensor("x", [TOK, D], f32, kind="ExternalInput")
    w_d = nc.dram_tensor("w", [D, IK], f32, kind="ExternalInput")
    out_d = nc.dram_tensor("out", [1, BC * IK], f32, kind="ExternalOutput")

    # x viewed per batch: [b, p, (q d)] with token s = 4p + q
    xv = x_d.rearrange("(b p q) d -> b p (q d)", p=128, q=QB)

    def bc_ap(ap, free_dims, extra_off=0):
        """Custom AP keeping `ap`'s partition dim, custom free dims."""
        return bass.AP(
            tensor=ap.tensor,
            offset=ap.offset + extra_off,
            ap=[list(ap.ap[0])] + [list(d) for d in free_dims],
        )

    with tile.TileContext(nc) as tc:
        with (
            tc.tile_pool(name="const", bufs=1) as const,
            tc.tile_pool(name="big", bufs=1) as big,
            tc.tile_pool(name="xin", bufs=4) as xin,
            tc.tile_pool(name="xtsb", bufs=2) as xtsb,
            tc.tile_pool(name="xtps", bufs=2, space="PSUM") as xtps,
            tc.tile_pool(name="ups", bufs=2, space="PSUM") as ups,
            tc.tile_pool(name="ops", bufs=2, space="PSUM") as ops_pool,
        ):
            # ---- constants ----
            w_sb = const.tile([128, IK], f32)
            nc.sync.dma_start(out=w_sb[:D, :], in_=w_d[:, :])
            ident = const.tile([128, 128], f32)
            make_identity(nc, ident[:])
            ones = const.tile([128, 128], f32)
            nc.vector.memset(ones[:], 1.0)

            # ---- persistent big tensors ----
            u1 = big.tile([128, QB, BC, NCAP, DCAP], f32)   # (q, b, i, k)
            cu = big.tile([128, QB, BC, NCAP, DCAP], f32)   # cu / uo scratch
            o2 = big.tile([128, BC, NCAP, DCAP], f32)        # o replicated (b,i,k)
            sqt = big.tile([128, BC, NCAP, DCAP], f32)       # squares / final
            blog = big.tile([128, QB, BC, NCAP], f32)        # (q, b, i)
            e_t = big.tile([128, QB, BC, NCAP], f32)
            c_t = big.tile([128, QB, BC, NCAP], f32)
            z_t = big.tile([128, QB, BC], f32)
            zinv = big.tile([128, QB, BC], f32)
            ss = big.tile([128, BC, NCAP], f32)
            inv = big.tile([128, BC, NCAP], f32)
            r_t = big.tile([128, BC, NCAP], f32)

            u1_flat = u1[:]          # full AP [128, 12800]
            cu_flat = cu[:]

            # ================= Phase 1: u_hat GEMM =================
            for g in range(NQUAD):
                u_ps = ups.tile([128, CHUNK * IK], f32)  # [128, 400]
                for h in range(2):  # two xt-groups of 2 batches each
                    xt_ps = xtps.tile([128, 1024], f32)
                    xt_sb = xtsb.tile([128, 1024], f32)
                    for bb in range(2):
                        b = g * QB + h * 2 + bb
                        x_sb = xin.tile([128, QB * D], f32)
                        nc.sync.dma_start(out=x_sb[:], in_=xv[b])
                        for q in range(QB):
                            t = bb * QB + q
                            nc.tensor.transpose(
                                xt_ps[:D, t * 128:(t + 1) * 128],
                                x_sb[:, q * D:(q + 1) * D],
                                ident[:],
                            )
                    nc.scalar.copy(out=xt_sb[:D, :], in_=xt_ps[:D, :])
                    for t in range(8):
                        tt = h * 8 + t
                        nc.tensor.matmul(
                            u_ps[:, tt * IK:(tt + 1) * IK],
                            xt_sb[:D, t * 128:(t + 1) * 128],
                            w_sb[:D, :],
                            start=True, stop=True,
                        )
                # scatter 16 (bq, q) tiles into u1[(q, b, ...)] layout
                # psum columns enumerate (bq, q, ik); dest offsets:
                #   bq: step IK (consecutive batches), q: step BC*IK, ik: 1
                dest = bc_ap(
                    u1_flat,
                    [[IK, QB], [BC * IK, QB], [1, IK]],
                    extra_off=g * QB * IK,
                )
                nc.vector.tensor_copy(out=dest, in_=u_ps[:])

            # ================= Phase 2: routing =================
            for it in range(ROUTINGS):
                src = u1 if it == 0 else cu
                for ci in range(NCHUNK):
                    o_ps = ops_pool.tile([128, CHUNK * IK], f32)
                    for q in range(QB):
                        nc.tensor.matmul(
                            o_ps[:],
                            ones[:],
                            src[:, q, ci * CHUNK:(ci + 1) * CHUNK, :, :],
                            start=(q == 0), stop=(q == QB - 1),
                        )
                    # copy (replicated) chunk into o2
                    nc.scalar.copy(
                        out=o2[:, ci * CHUNK:(ci + 1) * CHUNK, :, :],
                        in_=o_ps[:],
                    )
                # squash stats: ss = sum_k o^2 ; r = 1/sqrt(ss + eps)
                nc.vector.tensor_mul(sqt[:], o2[:], o2[:])
                nc.vector.reduce_sum(ss[:], sqt[:], axis=mybir.AxisListType.X)
                nc.vector.tensor_scalar_add(inv[:], ss[:], EPS)
                nc.vector.reciprocal(inv[:], inv[:])
                nc.scalar.sqrt(r_t[:], inv[:])

                if it < ROUTINGS - 1:
                    # blog[q,b,i] = r[b,i] * sum_k u1[q,b,i,k]*o2[b,i,k]
                    o2_bc = bc_ap(o2[:], [[0, QB], [1, BC * IK]])
                    nc.vector.tensor_mul(cu_flat, u1_flat, o2_bc)
                    nc.vector.reduce_sum(
                        blog[:], cu[:], axis=mybir.AxisListType.X
                    )
                    r_bc = bc_ap(r_t[:], [[0, QB], [1, BC * NCAP]])
                    nc.vector.tensor_mul(blog[:], blog[:], r_bc)
                    # softmax over i
                    nc.scalar.activation(out=e_t[:], in_=blog[:], func=AF.Exp)
                    nc.vector.reduce_sum(
                        z_t[:], e_t[:], axis=mybir.AxisListType.X
                    )
                    nc.vector.reciprocal(zinv[:], z_t[:])
                    zinv_bc = bc_ap(zinv[:], [[1, QB * BC], [0, NCAP]])
                    nc.vector.tensor_mul(c_t[:], e_t[:], zinv_bc)
                    # cu[q,b,i,k] = c[q,b,i] * u1[q,b,i,k]
                    c_bc = bc_ap(c_t[:], [[1, QB * BC * NCAP], [0, DCAP]])
                    nc.vector.tensor_mul(cu_flat, u1_flat, c_bc)
                else:
                    # final normalize: out = o2 * r (broadcast over k)
                    r_bck = bc_ap(r_t[:], [[1, BC * NCAP], [0, DCAP]])
                    nc.vector.tensor_mul(sqt[:], o2[:], r_bck)
                    nc.sync.dma_start(
                        out=out_d[:, :],
                        in_=sqt[0:1, :, :, :],
                    )
    nc.compile()
    return nc


_NC = None


def kernel(x: np.ndarray, W: np.ndarray) -> np.ndarray:
    from concourse.bass_utils import run_bass_kernel_spmd

    global _NC
    if _NC is None:
        _NC = _build()

    x = np.ascontiguousarray(x, dtype=np.float32)
    w = np.ascontiguousarray(W.reshape(D, IK), dtype=np.float32)
    xs = x.reshape(NCORES, TOK, D)
    in_maps = [{"x": xs[i], "w": w} for i in range(NCORES)]
    res = run_bass_kernel_spmd(_NC, in_maps, core_ids=list(range(NCORES)))
    out = np.concatenate(
        [r["out"].reshape(BC, NCAP, DCAP) for r in res.results], axis=0
    )
    return out


if __name__ == "__main__":
    rng = np.random.default_rng(0)
    x = rng.standard_normal((B, S, D), dtype=np.float32)
    W = rng.standard_normal((1, D, IK), dtype=np.float32) * 0.1
    out = kernel(x, W)
    print(out.shape, out.dtype)


# revision 19
# speedup vs baseline: 324.3877x; 324.3877x over previous
"""Trainium2 Bass kernel for nn_Caps_Layer (capsule routing layer).

Reference computation (per batch b of 1024):
  u_hat[b] = (x[b] @ W).reshape(512, 5, 5) -> [5cap, 512, 5dim]
  4 rounds of routing:
    c = softmax_over_cap(blog); o = squash(sum_s c*u); blog = einsum(o, u)
  output: o [1024, 5, 5]

Sharding: pure data parallel over batch across 8 cores (128 batches/core).

Per-core design (token-position on SBUF partitions; s = 4p + q):
  - x shard [65536, 120] streamed per batch into [128p, 4*120] (contiguous
    DMA at full HBM bandwidth; the s-permutation is routing-invariant).
  - PE-transpose f32 tiles [128t,120d] -> [120,128] (PSUM), ACT copy to
    SBUF, f32 GEMM vs W -> exact u tiles [128t, 25] (f32 PSUM).
  - u2 in SBUF f32 (precision-critical), layout [128p, (k:5, q:4, b, i:5)].
  - Routing on-chip: sum_s via PE matmul with an all-ones stationary
    [128,128] -> column sums replicated across all partitions (free
    partition-broadcast for the next product). Products (cu, uo) are
    written fp16 (measured absmax ~1.4e-3 vs f32 reference); softmax /
    squash statistics in f32 where it matters; fp16 2x DVE modes and
    1 cyc/row fp16 PE matmuls on the hot loops.
  - Two independent batch-halves so PE/ACT work of one half overlaps the
    serial DVE chain of the other, and phase 2 of half 0 overlaps
    phase 1 of half 1.
"""

import numpy as np

NCORES = 8
B, S, D = 1024, 512, 120
NCAP, DCAP = 5, 5
IK = NCAP * DCAP  # 25
BC = B // NCORES  # 128 batches per core
TOK = BC * S
EPS = 1e-7
ROUTINGS = 4

QB = 4             # s-phases per partition (s = 4p + q)
NHALF = 2
BH = BC // NHALF   # 64 batches per half
NQUAD_H = BH // 4  # 16 GEMM quads per half
CHUNK = 16         # batches per routing psum chunk
NCHUNK_H = BH // CHUNK  # 4


def _build(n_routing=ROUTINGS, do_phase2=True, precise=False):
    import concourse.bass as bass
    import concourse.bacc as bacc
    import concourse.tile as tile
    from concourse import mybir
    from concourse.masks import make_identity

    f32 = mybir.dt.float32
    rdt = mybir.dt.float32 if precise else mybir.dt.float16
    AF = mybir.ActivationFunctionType

    nc = bacc.Bacc("TRN2", target_bir_lowering=False, debug=False)
    x_d = nc.dram_tensor("x", [TOK, D], f32, kind="ExternalInput")
    w_d = nc.dram_tensor("w", [D, IK], f32, kind="ExternalInput")
    out_d = nc.dram_tensor("out", [1, BC * IK], f32, kind="ExternalOutput")

    xv = x_d.rearrange("(b p q) d -> b p (q d)", p=128, q=QB)

    def ap_of(tile_ap, free_dims, extra_off=0):
        return bass.AP(
            tensor=tile_ap.tensor,
            offset=tile_ap.offset + extra_off,
            ap=[list(tile_ap.ap[0])] + [list(d) for d in free_dims],
        )

    with tile.TileContext(nc) as tc:
        with (
            tc.tile_pool(name="const", bufs=1) as const,
            tc.tile_pool(name="big", bufs=1) as big,
            tc.tile_pool(name="xin", bufs=8) as xin,
            tc.tile_pool(name="xtsb", bufs=4) as xtsb,
            tc.tile_pool(name="xtps", bufs=2, space="PSUM") as xtps,
            tc.tile_pool(name="ups", bufs=2, space="PSUM") as ups,
            tc.tile_pool(name="ops", bufs=2, space="PSUM") as ops_pool,
        ):
            # ---- constants ----
            w_sb = const.tile([128, IK], f32)
            nc.sync.dma_start(out=w_sb[:D, :], in_=w_d[:, :])
            ident = const.tile([128, 128], f32)
            make_identity(nc, ident[:])
            ones = const.tile([128, 128], rdt)
            nc.vector.memset(ones[:], 1.0)
            ones32 = const.tile([128, 128], f32)
            nc.vector.memset(ones32[:], 1.0)
            eps_t = const.tile([128, 1], f32)
            nc.vector.memset(eps_t[:], EPS)

            # ---- per-half persistent tensors ----
            FH = QB * BH * IK  # 6400 elems/partition per half
            u2 = [big.tile([128, DCAP, QB, BH, NCAP], f32, name=f"u2_{h}")
                  for h in range(NHALF)]       # (k, q, b, i), f32 (precision)
            cu = [big.tile([128, FH], rdt, name=f"cu_{h}")
                  for h in range(NHALF)]       # cu (k,q,b,i) / uo (k,q,b,i)
            o2 = [big.tile([128, DCAP, BH, NCAP], rdt, name=f"o2_{h}")
                  for h in range(NHALF)]       # (k, b, i) replicated
            blog = [big.tile([128, QB, BH, NCAP], rdt, name=f"blog_{h}")
                    for h in range(NHALF)]     # (q, b, i)
            e_t = [big.tile([128, QB, BH, NCAP], rdt, name=f"e_{h}")
                   for h in range(NHALF)]
            c_t = [big.tile([128, QB, BH, NCAP], rdt, name=f"c_{h}")
                   for h in range(NHALF)]
            z_t = [big.tile([128, QB, BH], f32, name=f"z_{h}")
                   for h in range(NHALF)]
            zinv = [big.tile([128, QB, BH], f32, name=f"zi_{h}")
                    for h in range(NHALF)]
            sq_t = [big.tile([128, DCAP, BH, NCAP], rdt, name=f"sq_{h}")
                    for h in range(NHALF)]     # squares (k, b, i)
            ssp = [big.tile([128, 4, BH, NCAP], f32, name=f"ssp_{h}")
                   for h in range(NHALF)]      # partial sums scratch
            ss = [big.tile([128, BH, NCAP], f32, name=f"ss_{h}")
                  for h in range(NHALF)]
            rr = [big.tile([128, BH, NCAP], f32, name=f"rr_{h}")
                  for h in range(NHALF)]
            r16 = [big.tile([128, BH, NCAP], rdt, name=f"r16_{h}")
                   for h in range(NHALF)]
            fin = [big.tile([1, BH * IK], f32, name=f"fin_{h}")
                   for h in range(NHALF)]

            # ================= Phase 1: u_hat GEMM =================
            def phase1_half(h):
                u2h = u2[h][:]
                for gl in range(NQUAD_H):
                    u_ps = ups.tile([128, 16 * IK], f32, name="u_ps")
                    for hh in range(2):
                        xt_ps = xtps.tile([128, 1024], f32, name="xt_ps")
                        xt_sb = xtsb.tile([128, 1024], f32, name="xt_sb")
                        for bb in range(2):
                            b = h * BH + gl * 4 + hh * 2 + bb
                            x_sb = xin.tile([128, QB * D], f32, name="x_sb")
                            nc.sync.dma_start(out=x_sb[:], in_=xv[b])
                            for q in range(QB):
                                t = bb * QB + q
                                nc.tensor.transpose(
                                    xt_ps[:D, t * 128:(t + 1) * 128],
                                    x_sb[:, q * D:(q + 1) * D],
                                    ident[:],
                                )
                        nc.scalar.copy(out=xt_sb[:D, :], in_=xt_ps[:D, :])
                        for t in range(8):
                            tt = hh * 8 + t
                            nc.tensor.matmul(
                                u_ps[:, tt * IK:(tt + 1) * IK],
                                xt_sb[:D, t * 128:(t + 1) * 128],
                                w_sb[:D, :],
                                start=True, stop=True,
                            )
                    # scatter psum (bq, q, i, k) -> u2 (k, q, b, i); one
                    # copy per k (AP dim limit)
                    for k in range(DCAP):
                        src = ap_of(u_ps[:], [[100, 4], [25, 4], [5, 5]],
                                    extra_off=k)
                        dst = ap_of(u2h, [[NCAP, 4], [BH * NCAP, QB], [1, NCAP]],
                                    extra_off=k * QB * BH * NCAP + gl * 4 * NCAP)
                        nc.vector.tensor_copy(out=dst, in_=src)

            # ================= Phase 2: routing =================
            KSTR = QB * BH * NCAP  # u2/cu k-plane stride (1280)

            def routing_iter(h, it):
                u2h, cuh, o2h = u2[h][:], cu[h][:], o2[h][:]
                last = it == n_routing - 1
                src = u2h if it == 0 else cuh
                for ci in range(NCHUNK_H):
                    o_ps = ops_pool.tile([128, CHUNK * IK], f32, name="o_ps")
                    for q in range(QB):
                        rhs = ap_of(
                            src,
                            [[NCAP, CHUNK], [1, NCAP], [KSTR, DCAP]],
                            extra_off=q * BH * NCAP + ci * CHUNK * NCAP,
                        )
                        nc.tensor.matmul(
                            o_ps[:], ones32[:] if it == 0 else ones[:], rhs,
                            start=(q == 0), stop=(q == QB - 1),
                        )
                    # psum (b,i,k) -> o2 (k,b,i), cast to rdt
                    dst = ap_of(
                        o2h, [[NCAP, CHUNK], [1, NCAP], [BH * NCAP, DCAP]],
                        extra_off=ci * CHUNK * NCAP,
                    )
                    nc.scalar.copy(out=dst, in_=o_ps[:])
                # squash stats: ss = sum_k o^2 (k-plane tree) -> r
                sqh, sph = sq_t[h][:], ssp[h]
                nc.vector.tensor_mul(sqh, o2h, o2h)
                PL = BH * NCAP  # 320
                kpl = lambda k: ap_of(sqh, [[1, PL]], extra_off=k * PL)
                nc.vector.tensor_add(sph[:, 0], kpl(0), kpl(1))
                nc.vector.tensor_add(sph[:, 1], kpl(2), kpl(3))
                nc.vector.tensor_add(sph[:, 2], sph[:, 0], sph[:, 1])
                nc.vector.tensor_add(ss[h][:], sph[:, 2], kpl(4))
                nc.scalar.activation(
                    out=rr[h][:], in_=ss[h][:], func=AF.Sqrt, bias=eps_t[:],
                )
                nc.vector.reciprocal(rr[h][:], rr[h][:])
                if not last:
                    nc.vector.tensor_copy(out=r16[h][:], in_=rr[h][:])
                    # uo = u2 * o2_bcast  (both (k,q,b,i)); reuse cu buffer
                    o2_bc = ap_of(
                        o2h, [[PL, DCAP], [0, QB], [1, PL]])
                    nc.vector.tensor_mul(cuh, u2h, o2_bc)
                    # blog = sum_k uo (in-place accumulate over k-planes)
                    bl = blog[h][:]
                    uo_k = lambda k: ap_of(cuh, [[1, KSTR]], extra_off=k * KSTR)
                    nc.vector.tensor_add(bl, uo_k(0), uo_k(1))
                    nc.vector.tensor_add(bl, bl, uo_k(2))
                    nc.vector.tensor_add(bl, bl, uo_k(3))
                    nc.vector.tensor_add(bl, bl, uo_k(4))
                    # fold in r (temperature) and softmax over i
                    r_bc = ap_of(r16[h][:], [[0, QB], [1, PL]])
                    nc.vector.tensor_mul(bl, bl, r_bc)
                    nc.scalar.activation(out=e_t[h][:], in_=bl, func=AF.Exp)
                    nc.vector.reduce_sum(
                        z_t[h][:], e_t[h][:], axis=mybir.AxisListType.X)
                    nc.vector.reciprocal(zinv[h][:], z_t[h][:])
                    zi_bc = ap_of(zinv[h][:], [[1, QB * BH], [0, NCAP]])
                    nc.vector.tensor_mul(c_t[h][:], e_t[h][:], zi_bc)
                    # cu = u2 * c_bcast(k)
                    c_bc = ap_of(c_t[h][:], [[0, DCAP], [1, KSTR]])
                    nc.vector.tensor_mul(cuh, u2h, c_bc)
                else:
                    # fin[(b,i,k)] = o2[(k,b,i)] * r  (row 0; all rows equal)
                    o2_row = bass.AP(
                        tensor=o2h.tensor, offset=o2h.offset,
                        ap=[[o2h.ap[0][0], 1], [NCAP, BH], [1, NCAP], [PL, DCAP]],
                    )
                    r_row = bass.AP(
                        tensor=rr[h][:].tensor, offset=rr[h][:].offset,
                        ap=[[rr[h][:].ap[0][0], 1], [NCAP, BH], [1, NCAP],
                            [0, DCAP]],
                    )
                    nc.vector.tensor_mul(fin[h][:], o2_row, r_row)
                    nc.sync.dma_start(
                        out=out_d[:, h * BH * IK:(h + 1) * BH * IK],
                        in_=fin[h][:],
                    )

            phase1_half(0)
            phase1_half(1)
            if do_phase2:
                for it in range(n_routing):
                    routing_iter(0, it)
                    routing_iter(1, it)
    nc.compile()
    return nc


_NC = None


def kernel(x: np.ndarray, W: np.ndarray) -> np.ndarray:
    from concourse.bass_utils import run_bass_kernel_spmd

    global _NC
    if _NC is None:
        _NC = _build()

    x = np.ascontiguousarray(x, dtype=np.float32)
    w = np.ascontiguousarray(W.reshape(D, IK), dtype=np.float32)
    xs = x.reshape(NCORES, TOK, D)
    in_maps = [{"x": xs[i], "w": w} for i in range(NCORES)]
    res = run_bass_kernel_spmd(_NC, in_maps, core_ids=list(range(NCORES)))
    out = np.concatenate(
        [r["out"].reshape(BC, NCAP, DCAP) for r in res.results], axis=0
    )
    return out


if __name__ == "__main__":
    rng = np.random.default_rng(0)
    x = rng.standard_normal((B, S, D), dtype=np.float32)
    W = rng.standard_normal((1, D, IK), dtype=np.float32) * 0.1
    out = kernel(x, W)
    print(out.shape, out.dtype)
